# revision 1
# baseline (speedup 1.0000x reference)
"""Trainium2 Bass kernel for nn_EnhancedTarotInterpreter (dense transformer decoder).

Sharding: pure data parallel over batch (16 -> 8 cores x 2). Each core runs the
full model on its 2 batch elements; no collectives. Inside a core:

- Activations are feature-major ("x.T": [d_chunk 128, tokens 2048]) in f32r so
  every dense matmul's lhsT is a weight chunk loaded transposed from DRAM.
- Cross-attention memory has length 1 -> softmax is identity -> the whole block
  collapses to one bias vector per batch element (precomputed in the prologue).
- Self-attention: scores transposed [t, s] (K=32 matmuls), exp straight out of
  PSUM on ACT (no max subtraction), causal mask only on the diagonal 128x128
  block (precomputed bf16 mask, DVE multiply), AV flipped (out [s,33] bf16)
  with a ones-column in V so the denominator lands per-partition.
- LayerNorm feature-major: column stats via ones-matmul on PE, per-column
  affine via PE rank-1 broadcasts into PSUM + two DVE passes.
- Final projection in bf16, out_w transposed on the DMA engines (cast-DMA to
  DRAM bf16, then XBAR DMA-transpose), streamed in vocab slabs.
"""

import sys

sys.path.insert(0, "/opt/trn_rl_repo")

import numpy as np

import concourse.bass as bass
import concourse.bacc as bacc
import concourse.mybir as mybir
import concourse.tile as tile
from concourse.bass_utils import run_bass_kernel_spmd

FP32 = mybir.dt.float32
FP32R = mybir.dt.float32r
BF16 = mybir.dt.bfloat16
I32 = mybir.dt.int32
AF = mybir.ActivationFunctionType
OP = mybir.AluOpType
AX = mybir.AxisListType

B, S, E, D, V, H, NL = 16, 1024, 64, 256, 10000, 8, 3
HD = D // H          # 32
FF = 4 * D           # 1024
NCORES = 8
BL = B // NCORES     # 2
S2 = BL * S          # 2048
VP = 10016           # vocab padded to 32 for DMA transpose
VSLAB = 1280         # vocab slab for the final projection
ISCL = 1.0 / float(np.sqrt(HD))
EPS = 1e-5

_CACHE = {}


def _t_ap(dram, offset, pstep, pcount, fstep, fcount):
    """Manual 2D access pattern on a DRAM tensor: [partition, free]."""
    h = dram.tensor if hasattr(dram, "tensor") else dram
    if pcount == 1 and pstep == 0:
        pstep = 1
    return bass.AP(tensor=h, offset=offset, ap=[[pstep, pcount], [fstep, fcount]])


def build():
    nc = bacc.Bacc("TRN2", target_bir_lowering=False)

    # ---------------- DRAM I/O ----------------
    tract = nc.dram_tensor("tractovka", [BL, E], FP32, kind="ExternalInput")
    ctx = nc.dram_tensor("context", [BL, E], FP32, kind="ExternalInput")
    card = nc.dram_tensor("card", [BL, E], FP32, kind="ExternalInput")
    enc_w = nc.dram_tensor("enc_w", [3, D, E], FP32, kind="ExternalInput")
    enc_b = nc.dram_tensor("enc_b", [3, D], FP32, kind="ExternalInput")
    enc_ln_g = nc.dram_tensor("enc_ln_g", [3, D], FP32, kind="ExternalInput")
    enc_ln_b = nc.dram_tensor("enc_ln_b", [3, D], FP32, kind="ExternalInput")
    fusion_w = nc.dram_tensor("fusion_w", [D, 3 * D], FP32, kind="ExternalInput")
    fusion_b = nc.dram_tensor("fusion_b", [D], FP32, kind="ExternalInput")
    fusion_ln_g = nc.dram_tensor("fusion_ln_g", [D], FP32, kind="ExternalInput")
    fusion_ln_b = nc.dram_tensor("fusion_ln_b", [D], FP32, kind="ExternalInput")
    tok_emb = nc.dram_tensor("tok_emb", [V, D], FP32, kind="ExternalInput")
    pos_emb = nc.dram_tensor("pos_emb", [1024, D], FP32, kind="ExternalInput")
    sa_in_w = nc.dram_tensor("sa_in_w", [NL, 3 * D, D], FP32, kind="ExternalInput")
    sa_in_b = nc.dram_tensor("sa_in_b", [NL, 3 * D], FP32, kind="ExternalInput")
    sa_out_w = nc.dram_tensor("sa_out_w", [NL, D, D], FP32, kind="ExternalInput")
    sa_out_b = nc.dram_tensor("sa_out_b", [NL, D], FP32, kind="ExternalInput")
    ca_in_w = nc.dram_tensor("ca_in_w", [NL, 3 * D, D], FP32, kind="ExternalInput")
    ca_in_b = nc.dram_tensor("ca_in_b", [NL, 3 * D], FP32, kind="ExternalInput")
    ca_out_w = nc.dram_tensor("ca_out_w", [NL, D, D], FP32, kind="ExternalInput")
    ca_out_b = nc.dram_tensor("ca_out_b", [NL, D], FP32, kind="ExternalInput")
    ln_g = [nc.dram_tensor(f"ln{k}_g", [NL, D], FP32, kind="ExternalInput") for k in (1, 2, 3)]
    ln_b = [nc.dram_tensor(f"ln{k}_b", [NL, D], FP32, kind="ExternalInput") for k in (1, 2, 3)]
    ffn_w1 = nc.dram_tensor("ffn_w1", [NL, FF, D], FP32, kind="ExternalInput")
    ffn_b1 = nc.dram_tensor("ffn_b1", [NL, FF], FP32, kind="ExternalInput")
    ffn_w2 = nc.dram_tensor("ffn_w2", [NL, D, FF], FP32, kind="ExternalInput")
    ffn_b2 = nc.dram_tensor("ffn_b2", [NL, D], FP32, kind="ExternalInput")
    out_w = nc.dram_tensor("out_w", [V, D], FP32, kind="ExternalInput")
    out_b = nc.dram_tensor("out_b", [V], FP32, kind="ExternalInput")
    prev = nc.dram_tensor("prev_tokens", [BL, S], I32, kind="ExternalInput")

    logits = nc.dram_tensor("logits", [S2, V], FP32, kind="ExternalOutput")
    outw_bf = nc.dram_tensor("outw_bf", [VP, D], BF16)   # internal
    outb_bf = nc.dram_tensor("outb_bf", [VP], BF16)      # internal

    from contextlib import ExitStack

    with tile.TileContext(nc) as tc:
        with ExitStack() as _es:
            P_const = _es.enter_context(tc.tile_pool(name="const", bufs=1))
            P_w = _es.enter_context(tc.tile_pool(name="wgt", bufs=1))
            P_stage = _es.enter_context(tc.tile_pool(name="stage", bufs=2))
            P_x = _es.enter_context(tc.tile_pool(name="X", bufs=4))
            P_qk = _es.enter_context(tc.tile_pool(name="qk", bufs=2))
            P_vex = _es.enter_context(tc.tile_pool(name="vex", bufs=16))
            P_e = _es.enter_context(tc.tile_pool(name="e", bufs=1))
            P_otok = _es.enter_context(tc.tile_pool(name="otok", bufs=9))
            P_oT = _es.enter_context(tc.tile_pool(name="oT", bufs=2))
            P_h1 = _es.enter_context(tc.tile_pool(name="h1", bufs=8))
            P_t1 = _es.enter_context(tc.tile_pool(name="t1", bufs=3))
            P_rows = _es.enter_context(tc.tile_pool(name="rows", bufs=2))
            P_small = _es.enter_context(tc.tile_pool(name="small", bufs=8))
            P_fin = _es.enter_context(tc.tile_pool(name="fin", bufs=2))
            PS_st = _es.enter_context(tc.tile_pool(name="psst", bufs=2, space="PSUM"))
            PS_mm = _es.enter_context(tc.tile_pool(name="psmm", bufs=4, space="PSUM"))

            def mmtile(shape=None, dtype=FP32):
                return PS_mm.tile([128, 512] if shape is None else shape, dtype,
                                  tag="mm", name="mm")

            # ---------------- constants ----------------
            ident_f = P_stage.tile([128, 128], FP32, tag="wstg", name="ident_f")
            nc.gpsimd.memset(ident_f[:], 0.0)
            nc.gpsimd.affine_select(
                out=ident_f[:], in_=ident_f[:], compare_op=OP.not_equal, fill=1.0,
                base=0, pattern=[[-1, 128]], channel_multiplier=1,
            )
            ident_r = P_const.tile([128, 128], FP32R)
            nc.vector.tensor_copy(ident_r[:], ident_f[:])
            ident_bf = P_const.tile([128, 128], BF16)
            nc.vector.tensor_copy(ident_bf[:], ident_f[:])

            masktri_f = P_stage.tile([128, 128], FP32, tag="wstg", name="masktri_f")
            nc.gpsimd.memset(masktri_f[:], 1.0)
            nc.gpsimd.affine_select(
                out=masktri_f[:], in_=masktri_f[:], compare_op=OP.is_ge, fill=0.0,
                base=0, pattern=[[1, 128]], channel_multiplier=-1,
            )
            masktri = P_const.tile([128, 128], BF16)
            nc.vector.tensor_copy(masktri[:], masktri_f[:])

            ones_f = P_const.tile([128, 1], FP32)
            nc.vector.memset(ones_f[:], 1.0)
            ones_col = P_const.tile([128, 1], FP32R)       # [K=128, M=1] stats lhsT
            nc.vector.tensor_copy(ones_col[:], ones_f[:])
            onesr_f = P_stage.tile([1, 512], FP32, tag="wstg", name="onesr_f")
            nc.vector.memset(onesr_f[:], 1.0)
            ones_row = P_const.tile([1, 512], FP32R)       # rank-1 lhsT/rhs rows
            nc.vector.tensor_copy(ones_row[:], onesr_f[:])
            ones_row_bf = P_const.tile([1, 128], BF16)
            nc.vector.tensor_copy(ones_row_bf[:], onesr_f[0:1, 0:128])

            eps1 = P_const.tile([1, 1], FP32)
            nc.vector.memset(eps1[:], EPS)
            eps2 = P_const.tile([BL, 1], FP32)
            nc.vector.memset(eps2[:], EPS)
            eps128 = P_const.tile([128, 1], FP32)
            nc.vector.memset(eps128[:], EPS)

            # ---------------- out_w / out_b path (DMA engines only) --------
            nc.gpsimd.dma_start(outw_bf[0:V, :], out_w[:])    # cast fp32->bf16
            nc.gpsimd.dma_start(outb_bf[0:V], out_b[:])       # cast fp32->bf16

            # ---------------- embedding -> xT ----------------
            xT = [P_x.tile([128, S2], FP32R, tag="X", name="xT") for _ in range(2)]
            for ti in range(16):
                b_, a_ = ti // 8, ti % 8
                idx_t = P_small.tile([128, 1], I32, tag="idx", name="idx")
                nc.sync.dma_start(idx_t[:], prev[b_, 128 * a_:128 * (a_ + 1)])
                emb_t = P_stage.tile([128, D], FP32, tag="emb", name="emb")
                nc.gpsimd.indirect_dma_start(
                    out=emb_t[:], out_offset=None, in_=tok_emb[:],
                    in_offset=bass.IndirectOffsetOnAxis(ap=idx_t[:, 0:1], axis=0),
                )
                pos_t = P_stage.tile([128, D], FP32, tag="pos", name="pos")
                nc.sync.dma_start(pos_t[:], pos_emb[128 * a_:128 * (a_ + 1), :])
                sum_t = P_stage.tile([128, D], FP32R, tag="esum", name="esum")
                nc.vector.tensor_tensor(out=sum_t[:], in0=emb_t[:], in1=pos_t[:], op=OP.add)
                for c in range(2):
                    pt = mmtile([128, 128], FP32R)
                    nc.tensor.transpose(pt[:], sum_t[:, 128 * c:128 * (c + 1)], ident_r[:])
                    nc.vector.tensor_copy(xT[c][:, 128 * ti:128 * (ti + 1)], pt[:])

            # ---------------- encoders / fusion / cross-attn vectors -------
            def token_ln_gelu(psum_ap, gb_off, g_src, b_src, do_gelu):
                red = P_small.tile([BL, 1], FP32, tag="red", name="red")
                nc.vector.tensor_reduce(out=red[:], in_=psum_ap, axis=AX.X, op=OP.add)
                m = P_small.tile([BL, 1], FP32, tag="m", name="m")
                nc.vector.tensor_scalar(out=m[:], in0=red[:], scalar1=1.0 / D,
                                        scalar2=None, op0=OP.mult)
                xc = P_stage.tile([BL, D], FP32, tag="xc", name="xc", bufs=1)
                nc.vector.tensor_scalar(out=xc[:], in0=psum_ap, scalar1=m[:],
                                        scalar2=None, op0=OP.subtract)
                sq = P_stage.tile([BL, D], FP32, tag="sq", name="sq", bufs=1)
                nc.vector.tensor_tensor(out=sq[:], in0=xc[:], in1=xc[:], op=OP.mult)
                red2 = P_small.tile([BL, 1], FP32, tag="red2", name="red2")
                nc.vector.tensor_reduce(out=red2[:], in_=sq[:], axis=AX.X, op=OP.add)
                var = P_small.tile([BL, 1], FP32, tag="var", name="var")
                nc.vector.tensor_scalar(out=var[:], in0=red2[:], scalar1=1.0 / D,
                                        scalar2=None, op0=OP.mult)
                std = P_small.tile([BL, 1], FP32, tag="std", name="std")
                nc.scalar.activation(std[:], var[:], AF.Sqrt, bias=eps2[:], scale=1.0)
                rstd = P_small.tile([BL, 1], FP32, tag="rstd", name="rstd")
                nc.vector.reciprocal(rstd[:], std[:])
                xn = P_stage.tile([BL, D], FP32, tag="xn", name="xn", bufs=1)
                nc.vector.tensor_scalar(out=xn[:], in0=xc[:], scalar1=rstd[:],
                                        scalar2=None, op0=OP.mult)
                gb = P_stage.tile([BL, D], FP32, tag="gbb", name="gb")
                nc.sync.dma_start(gb[:], _t_ap(g_src, gb_off, 0, BL, 1, D))
                nc.vector.tensor_tensor(out=xn[:], in0=xn[:], in1=gb[:], op=OP.mult)
                bb = P_stage.tile([BL, D], FP32, tag="gbb", name="bb")
                nc.sync.dma_start(bb[:], _t_ap(b_src, gb_off, 0, BL, 1, D))
                out_t = P_stage.tile([BL, D], FP32, tag="encout", name="encout", bufs=4)
                if do_gelu:
                    nc.vector.tensor_tensor(out=xn[:], in0=xn[:], in1=bb[:], op=OP.add)
                    nc.scalar.activation(out_t[:], xn[:], AF.Gelu)
                else:
                    nc.vector.tensor_tensor(out=out_t[:], in0=xn[:], in1=bb[:], op=OP.add)
                return out_t

            def small_transposes(src_fp32, n_chunks, tag):
                src_r = P_stage.tile(list(src_fp32.shape), FP32R, tag="str",
                                     name="str", bufs=1)
                nc.vector.tensor_copy(src_r[:], src_fp32[:])
                outs = []
                for k in range(n_chunks):
                    pt = mmtile([128, BL], FP32R)
                    nc.tensor.transpose(
                        pt[:], src_r[0:BL, 128 * k:128 * (k + 1)], ident_r[0:BL, 0:BL]
                    )
                    st = P_small.tile([128, BL], FP32R, tag=tag, name=tag, bufs=8)
                    nc.vector.tensor_copy(st[:], pt[:])
                    outs.append(st)
                return outs

            def load_wT_f32r(dram, offset, fstep, fcount, tag, bufs=2):
                stg = P_stage.tile([128, fcount], FP32, tag="wstg", name="wstg")
                nc.sync.dma_start(stg[:], _t_ap(dram, offset, 1, 128, fstep, fcount))
                wt = P_w.tile([128, fcount], FP32R, tag=tag, name=tag, bufs=bufs)
                nc.gpsimd.tensor_copy(wt[:], stg[:])
                return wt

            def load_wT_bf16(dst_ap, dram, offset, fstep, fcount):
                """Transposed strided load -> fp32 staging -> POOL cast to bf16 dst."""
                done = 0
                while done < fcount:
                    fl = min(512, fcount - done)
                    stg = P_stage.tile([128, fl], FP32, tag="wstg", name="wstg")
                    nc.sync.dma_start(stg[:],
                                      _t_ap(dram, offset + done * fstep, 1, 128, fstep, fl))
                    nc.gpsimd.tensor_copy(dst_ap[:, done:done + fl], stg[:])
                    done += fl

            def row_f32r(dram, offset, n, tag, scale=None):
                rf = P_rows.tile([1, n], FP32, tag=tag + "f", name=tag + "f")
                nc.sync.dma_start(rf[:], _t_ap(dram, offset, 0, 1, 1, n))
                rr = P_rows.tile([1, n], FP32R, tag=tag, name=tag)
                if scale is None:
                    nc.vector.tensor_copy(rr[:], rf[:])
                else:
                    nc.vector.tensor_scalar(out=rr[:], in0=rf[:], scalar1=scale,
                                            scalar2=None, op0=OP.mult)
                return rr

            def col_fp32(dram, offset, tag, scale=None):
                cf = P_small.tile([128, 1], FP32, tag=tag, name=tag)
                nc.sync.dma_start(cf[:], _t_ap(dram, offset, 1, 128, 1, 1))
                if scale is not None:
                    nc.vector.tensor_scalar(out=cf[:], in0=cf[:], scalar1=scale,
                                            scalar2=None, op0=OP.mult)
                return cf

            enc_outs = []
            for i, src in enumerate((tract, ctx, card)):
                src_sb = P_stage.tile([BL, E], FP32, tag="encin", name="encin", bufs=1)
                nc.sync.dma_start(src_sb[:], src[:])
                src_r = P_stage.tile([BL, E], FP32R, tag="encinr", name="encinr", bufs=1)
                nc.vector.tensor_copy(src_r[:], src_sb[:])
                inT = mmtile([E, BL], FP32R)
                nc.tensor.transpose(inT[:], src_r[:], ident_r[0:BL, 0:BL])
                inT_sb = P_small.tile([E, BL], FP32R, tag="encT", name="encT", bufs=3)
                nc.vector.tensor_copy(inT_sb[:], inT[:])
                ewT_stg = P_stage.tile([E, D], FP32, tag="wstg", name="wstg")
                nc.sync.dma_start(ewT_stg[:], _t_ap(enc_w, i * D * E, 1, E, E, D))
                ewT = P_w.tile([E, D], FP32R, tag="encw", name="encw", bufs=2)
                nc.gpsimd.tensor_copy(ewT[:], ewT_stg[:])
                pe_ = mmtile([BL, D])
                nc.tensor.matmul(pe_[:], inT_sb[:], ewT[:], start=True, stop=False)
                ebr = row_f32r(enc_b, i * D, D, "smallrow")
                nc.tensor.matmul(pe_[:], ones_row[0:1, 0:BL], ebr[:], start=False, stop=True)
                enc_outs.append(token_ln_gelu(pe_[:], i * D, enc_ln_g, enc_ln_b, True))

            cat = P_stage.tile([BL, 3 * D], FP32, tag="cat", name="cat", bufs=1)
            for i in range(3):
                nc.vector.tensor_copy(cat[:, D * i:D * (i + 1)], enc_outs[i][:])
            catT = small_transposes(cat, 6, "catT")
            fwT = [load_wT_f32r(fusion_w, 128 * k, 3 * D, D, "fwT", bufs=3)
                   for k in range(6)]
            pf = mmtile([BL, D])
            for k in range(6):
                nc.tensor.matmul(pf[:], catT[k][:], fwT[k][:], start=(k == 0), stop=False)
            fbr = row_f32r(fusion_b, 0, D, "smallrow")
            nc.tensor.matmul(pf[:], ones_row[0:1, 0:BL], fbr[:], start=False, stop=True)
            mem = token_ln_gelu(pf[:], 0, fusion_ln_g, fusion_ln_b, True)

            memT = small_transposes(mem, 2, "memT")
            oca = []
            for i in range(NL):
                wv = [load_wT_f32r(ca_in_w, i * 3 * D * D + 2 * D * D + 128 * c, D, D,
                                   "cawv", bufs=2) for c in range(2)]
                pv = mmtile([BL, D])
                for c in range(2):
                    nc.tensor.matmul(pv[:], memT[c][:], wv[c][:], start=(c == 0), stop=False)
                vbr = row_f32r(ca_in_b, i * 3 * D + 2 * D, D, "smallrow")
                nc.tensor.matmul(pv[:], ones_row[0:1, 0:BL], vbr[:], start=False, stop=True)
                v_sb = P_stage.tile([BL, D], FP32, tag="cav", name="cav", bufs=1)
                nc.vector.tensor_copy(v_sb[:], pv[:])
                vT = small_transposes(v_sb, 2, "vT")
                wo = [load_wT_f32r(ca_out_w, i * D * D + 128 * c, D, D, "cawo", bufs=2)
                      for c in range(2)]
                po = mmtile([BL, D])
                for c in range(2):
                    nc.tensor.matmul(po[:], vT[c][:], wo[c][:], start=(c == 0), stop=False)
                obr = row_f32r(ca_out_b, i * D, D, "smallrow")
                nc.tensor.matmul(po[:], ones_row[0:1, 0:BL], obr[:], start=False, stop=True)
                o_sb = P_stage.tile([BL, D], FP32, tag="cao", name="cao", bufs=1)
                nc.vector.tensor_copy(o_sb[:], po[:])
                ocT = small_transposes(o_sb, 2, "ocT")
                ocf = []
                for c in range(2):
                    t = P_small.tile([128, BL], FP32, tag="oca", name="oca", bufs=6)
                    nc.vector.tensor_copy(t[:], ocT[c][:].bitcast(FP32))
                    ocf.append(t)
                oca.append(ocf)

            # ---------------- feature-major LayerNorm ----------------
            def layer_norm(xr, g_dram, b_dram, g_off):
                g_col = [col_fp32(g_dram, g_off + 128 * c, "lng") for c in range(2)]
                g_row = [row_f32r(g_dram, g_off + 128 * c, 128, "lngr") for c in range(2)]
                nb_row = [row_f32r(b_dram, g_off + 128 * c, 128, "lnbr", scale=-1.0)
                          for c in range(2)]
                # stat rows live at partitions {0,32,64,96} (32-aligned AP bases);
                # full-height ops cost free_size cycles regardless of partitions.
                m4 = P_rows.tile([128, 512], FP32, tag="m4", name="m4", bufs=1)
                e24 = P_rows.tile([128, 512], FP32, tag="e24", name="e24", bufs=1)
                msq4 = P_rows.tile([128, 512], FP32, tag="msq4", name="msq4", bufs=1)
                for j in range(4):
                    sl = slice(512 * j, 512 * (j + 1))
                    xsq = [P_t1.tile([128, 512], FP32R, tag="t1", name="xsq")
                           for _ in range(2)]
                    for c in range(2):
                        nc.vector.tensor_tensor(out=xsq[c][:], in0=xr[c][:, sl],
                                                in1=xr[c][:, sl], op=OP.mult)
                    st_ = mmtile()
                    nc.tensor.matmul(st_[0:1, :], ones_col[:], xr[0][:, sl],
                                     start=True, stop=False)
                    nc.tensor.matmul(st_[0:1, :], ones_col[:], xr[1][:, sl],
                                     start=False, stop=True)
                    st2_ = mmtile()
                    nc.tensor.matmul(st2_[0:1, :], ones_col[:], xsq[0][:],
                                     start=True, stop=False)
                    nc.tensor.matmul(st2_[0:1, :], ones_col[:], xsq[1][:],
                                     start=False, stop=True)
                    nc.vector.tensor_scalar(out=m4[32 * j:32 * j + 1, :], in0=st_[0:1, :],
                                            scalar1=1.0 / D, scalar2=None, op0=OP.mult)
                    nc.scalar.mul(e24[32 * j:32 * j + 1, :], st2_[0:1, :], 1.0 / D)
                nc.scalar.activation(msq4[:], m4[:], AF.Square)
                nc.vector.tensor_tensor(out=e24[:], in0=e24[:], in1=msq4[:],
                                        op=OP.subtract)
                nc.scalar.activation(e24[:], e24[:], AF.Sqrt, bias=eps128[:], scale=1.0)
                nc.vector.reciprocal(e24[:], e24[:])   # e24 now holds rstd rows
                xo = [P_x.tile([128, S2], FP32R, tag="X", name="xo") for _ in range(2)]
                for j in range(4):
                    sl = slice(512 * j, 512 * (j + 1))
                    r_r = P_rows.tile([1, 512], FP32R, tag="rr", name="rr", bufs=2)
                    nc.vector.tensor_copy(r_r[:], e24[32 * j:32 * j + 1, :])
                    c_r = P_rows.tile([1, 512], FP32R, tag="cr", name="cr", bufs=2)
                    nc.vector.tensor_tensor(out=c_r[:], in0=m4[32 * j:32 * j + 1, :],
                                            in1=e24[32 * j:32 * j + 1, :], op=OP.mult)
                    rb = mmtile()
                    nc.tensor.matmul(rb[:], ones_row[0:1, 0:128], r_r[:],
                                     start=True, stop=True)
                    for c in range(2):
                        db = mmtile()
                        nc.tensor.matmul(db[:], g_row[c][:], c_r[:],
                                         start=True, stop=False)
                        nc.tensor.matmul(db[:], nb_row[c][:], ones_row[:],
                                         start=False, stop=True)
                        t1 = P_t1.tile([128, 512], FP32, tag="t1", name="t1")
                        nc.vector.tensor_tensor(out=t1[:], in0=xr[c][:, sl], in1=rb[:],
                                                op=OP.mult)
                        nc.vector.scalar_tensor_tensor(
                            out=xo[c][:, sl], in0=t1[:], scalar=g_col[c][:], in1=db[:],
                            op0=OP.mult, op1=OP.subtract,
                        )
                return xo

            # ---------------- decoder layers ----------------
            x = xT
            for li in range(NL):
                wbase = li * 3 * D * D
                wInT = [load_wT_f32r(sa_in_w, wbase + 128 * c, D, 2 * D, f"wInT{c}",
                                     bufs=1) for c in range(2)]
                wvx_stg = [P_stage.tile([128, 264], FP32, tag="wvstg", name="wvstg")
                           for _ in range(2)]
                for c in range(2):
                    nc.vector.memset(wvx_stg[c][:], 0.0)
                    for h in range(H):
                        nc.sync.dma_start(
                            wvx_stg[c][:, 33 * h:33 * h + 32],
                            _t_ap(sa_in_w, wbase + (2 * D + 32 * h) * D + 128 * c,
                                  1, 128, D, 32),
                        )
                wvxT = [P_w.tile([128, 264], FP32R, tag=f"wvxT{c}", name=f"wvxT{c}",
                                 bufs=1) for c in range(2)]
                for c in range(2):
                    nc.gpsimd.tensor_copy(wvxT[c][:], wvx_stg[c][:])
                bx_f = P_rows.tile([1, 264], FP32, tag="bxf", name="bxf", bufs=1)
                nc.vector.memset(bx_f[:], 1.0)
                vb_stg = P_rows.tile([1, 256], FP32, tag="vbstg", name="vbstg", bufs=1)
                nc.sync.dma_start(vb_stg[:], _t_ap(sa_in_b, li * 3 * D + 2 * D, 0, 1, 1, D))
                for h in range(H):
                    nc.vector.tensor_copy(bx_f[0:1, 33 * h:33 * h + 32],
                                          vb_stg[0:1, 32 * h:32 * h + 32])
                bx_r = P_rows.tile([1, 264], FP32R, tag="bxr", name="bxr")
                nc.vector.tensor_copy(bx_r[:], bx_f[:])
                woT = [P_w.tile([128, D], BF16, tag=f"woT{c}", name=f"woT{c}", bufs=1)
                       for c in range(2)]
                for c in range(2):
                    load_wT_bf16(woT[c], sa_out_w, li * D * D + 128 * c, D, D)
                w1T = [P_w.tile([128, FF], FP32R, tag=f"w1T{c}", name=f"w1T{c}", bufs=1)
                       for c in range(2)]
                for c in range(2):
                    load_wT_bf16(w1T[c], ffn_w1, li * FF * D + 128 * c, D, FF)
                w2T = [P_w.tile([128, D], BF16, tag=f"w2T{k}", name=f"w2T{k}", bufs=1)
                       for k in range(8)]
                for k in range(8):
                    load_wT_bf16(w2T[k], ffn_w2, li * D * FF + 128 * k, FF, D)
                inb = [col_fp32(sa_in_b, li * 3 * D + 128 * oc, "inb",
                                scale=(ISCL if oc < 2 else None)) for oc in range(4)]
                ob_col = [col_fp32(sa_out_b, li * D + 128 * c, "obc") for c in range(2)]
                b1_col = [col_fp32(ffn_b1, li * FF + 128 * k, "b1c") for k in range(8)]
                b2_col = [col_fp32(ffn_b2, li * D + 128 * c, "b2c") for c in range(2)]

                # --- q,k projections (bf16; q pre-scaled by 1/sqrt(HD)) ---
                qT = [P_qk.tile([128, S2], BF16, tag="qT", name="qT") for _ in range(2)]
                kT = [P_qk.tile([128, S2], BF16, tag="kT", name="kT") for _ in range(2)]
                for oc in range(4):
                    dst = qT[oc] if oc < 2 else kT[oc - 2]
                    for j in range(4):
                        sl = slice(512 * j, 512 * (j + 1))
                        p = mmtile()
                        nc.tensor.matmul(p[:], wInT[0][:, 128 * oc:128 * (oc + 1)],
                                         x[0][:, sl], start=True, stop=False)
                        nc.tensor.matmul(p[:], wInT[1][:, 128 * oc:128 * (oc + 1)],
                                         x[1][:, sl], start=False, stop=True)
                        if oc < 2:
                            nc.vector.tensor_scalar(out=dst[:, sl], in0=p[:],
                                                    scalar1=inb[oc][:], scalar2=ISCL,
                                                    op0=OP.add, op1=OP.mult)
                        else:
                            nc.vector.tensor_scalar(out=dst[:, sl], in0=p[:],
                                                    scalar1=inb[oc][:], scalar2=None,
                                                    op0=OP.add)

                # --- v_ext [t, 264] bf16 ---
                vex = []
                for ti in range(16):
                    p = mmtile()
                    nc.tensor.matmul(p[:, 0:264], x[0][:, 128 * ti:128 * (ti + 1)],
                                     wvxT[0][:], start=True, stop=False)
                    nc.tensor.matmul(p[:, 0:264], x[1][:, 128 * ti:128 * (ti + 1)],
                                     wvxT[1][:], start=False, stop=False)
                    nc.tensor.matmul(p[:, 0:264], ones_row[0:1, 0:128], bx_r[:],
                                     start=False, stop=True)
                    vt = P_vex.tile([128, 264], BF16, tag="vex", name="vex")
                    nc.vector.tensor_copy(vt[:], p[:, 0:264])
                    vex.append(vt)

                # --- attention ---
                oT = [P_oT.tile([128, S2], BF16, tag="oT", name="oT") for _ in range(2)]
                for b_ in range(BL):
                    otoks = [P_otok.tile([128, 256], BF16, tag="otok", name="otok")
                             for _ in range(8)]
                    for h in range(H):
                        ch, po = h // 4, (h % 4) * 32
                        et = []
                        for a in range(8):
                            s0 = 128 * a
                            stp = PS_st.tile([128, 1024], FP32, tag="st", name="st")
                            breaks = [s0, 512, 1024] if s0 < 512 else [s0, 1024]
                            for cs, ce in zip(breaks[:-1], breaks[1:]):
                                nc.tensor.matmul(
                                    stp[:, cs:ce],
                                    kT[ch][po:po + 32, S * b_ + s0:S * b_ + s0 + 128],
                                    qT[ch][po:po + 32, S * b_ + cs:S * b_ + ce],
                                    start=True, stop=True,
                                    tile_position=(po, 0),
                                )
                            e_a = P_e.tile([128, 1024 - s0], BF16, tag=f"e{a}",
                                           name=f"e{a}")
                            nc.scalar.activation(e_a[:], stp[:, s0:1024], AF.Exp)
                            nc.vector.tensor_tensor(out=e_a[:, 0:128], in0=e_a[:, 0:128],
                                                    in1=masktri[:], op=OP.mult)
                            et.append(e_a)
                        for si in range(8):
                            pav = mmtile([128, 33])
                            for a in range(si + 1):
                                nc.tensor.matmul(
                                    pav[:],
                                    et[a][:, 128 * (si - a):128 * (si - a) + 128],
                                    vex[8 * b_ + a][:, 33 * h:33 * h + 33],
                                    start=(a == 0), stop=(a == si),
                                )
                            rr = P_small.tile([128, 1], FP32, tag="avrr", name="avrr")
                            nc.vector.reciprocal(rr[:], pav[:, 32:33])
                            nc.vector.tensor_scalar(
                                out=otoks[si][:, 32 * h:32 * h + 32], in0=pav[:, 0:32],
                                scalar1=rr[:], scalar2=None, op0=OP.mult,
                            )
                    for si in range(8):
                        for c in range(2):
                            pt = mmtile([128, 128], BF16)
                            nc.tensor.transpose(
                                pt[:], otoks[si][:, 128 * c:128 * (c + 1)], ident_bf[:]
                            )
                            nc.vector.tensor_copy(
                                oT[c][:, S * b_ + 128 * si:S * b_ + 128 * (si + 1)],
                                pt[:],
                            )

                # --- out_proj + residual -> xr1, ln1 -> x1 ---
                xr1 = [P_x.tile([128, S2], FP32R, tag="X", name="xr1") for _ in range(2)]
                for c in range(2):
                    for j in range(4):
                        sl = slice(512 * j, 512 * (j + 1))
                        p = mmtile()
                        nc.tensor.matmul(p[:], woT[0][:, 128 * c:128 * (c + 1)],
                                         oT[0][:, sl], start=True, stop=False)
                        nc.tensor.matmul(p[:], woT[1][:, 128 * c:128 * (c + 1)],
                                         oT[1][:, sl], start=False, stop=True)
                        nc.vector.scalar_tensor_tensor(
                            out=xr1[c][:, sl], in0=p[:], scalar=ob_col[c][:],
                            in1=x[c][:, sl], op0=OP.add, op1=OP.add,
                        )
                x1 = layer_norm(xr1, ln_g[0], ln_b[0], li * D)

                # --- cross-attention add -> xr2, ln2 -> x2 ---
                xr2 = [P_x.tile([128, S2], FP32R, tag="X", name="xr2") for _ in range(2)]
                for c in range(2):
                    for b_ in range(BL):
                        sl = slice(S * b_, S * (b_ + 1))
                        nc.vector.tensor_scalar(
                            out=xr2[c][:, sl], in0=x1[c][:, sl],
                            scalar1=oca[li][c][:, b_:b_ + 1], scalar2=None, op0=OP.add,
                        )
                x2 = layer_norm(xr2, ln_g[1], ln_b[1], li * D)

                # --- FFN -> xr3, ln3 -> x ---
                xr3 = [P_x.tile([128, S2], FP32R, tag="X", name="xr3") for _ in range(2)]
                for j in range(4):
                    sl = slice(512 * j, 512 * (j + 1))
                    h1t = []
                    for hk in range(8):
                        p = mmtile()
                        nc.tensor.matmul(p[:], w1T[0][:, 128 * hk:128 * (hk + 1)],
                                         x2[0][:, sl], start=True, stop=False)
                        nc.tensor.matmul(p[:], w1T[1][:, 128 * hk:128 * (hk + 1)],
                                         x2[1][:, sl], start=False, stop=True)
                        ht = P_h1.tile([128, 512], BF16, tag="h1", name="h1")
                        nc.scalar.activation(ht[:], p[:], AF.Relu, bias=b1_col[hk][:],
                                             scale=1.0)
                        h1t.append(ht)
                    for c in range(2):
                        p = mmtile()
                        for k in range(8):
                            nc.tensor.matmul(p[:], w2T[k][:, 128 * c:128 * (c + 1)],
                                             h1t[k][:], start=(k == 0), stop=(k == 7))
                        nc.vector.scalar_tensor_tensor(
                            out=xr3[c][:, sl], in0=p[:], scalar=b2_col[c][:],
                            in1=x2[c][:, sl], op0=OP.add, op1=OP.add,
                        )
                x = layer_norm(xr3, ln_g[2], ln_b[2], li * D)

            # ---------------- final projection (bf16, vocab slabs) ----------
            xb = [P_qk.tile([128, S2], BF16, tag="qT", name="xb") for _ in range(2)]
            for c in range(2):
                nc.vector.tensor_copy(xb[c][:], x[c][:])
            slab_edges = list(range(0, VP, VSLAB)) + [VP]  # 7x1280 + 1056
            for vq in range(len(slab_edges) - 1):
                v0q, v1q = slab_edges[vq], slab_edges[vq + 1]
                vw = v1q - v0q
                owq = [P_w.tile([128, VSLAB], BF16, tag=f"owq{c}", name=f"owq{c}",
                                bufs=2) for c in range(2)]
                for c in range(2):
                    nc.sync.dma_start(owq[c][:, 0:vw],
                                      outw_bf[v0q:v1q, 128 * c:128 * (c + 1)],
                                      transpose=True)
                obq = P_fin.tile([1, VSLAB], BF16, tag="obq", name="obq", bufs=1)
                nc.sync.dma_start(obq[0:1, 0:vw], outb_bf[v0q:v1q])
                real = min(v1q, V) - v0q
                for ti in range(16):
                    fst = P_fin.tile([128, VSLAB], FP32, tag="fst", name="fst", bufs=2)
                    nci = 0
                    for cs in range(0, vw, 512):
                        cl = min(512, vw - cs)
                        p = mmtile()
                        nc.tensor.matmul(p[:, 0:cl], xb[0][:, 128 * ti:128 * (ti + 1)],
                                         owq[0][:, cs:cs + cl], start=True, stop=False)
                        nc.tensor.matmul(p[:, 0:cl], xb[1][:, 128 * ti:128 * (ti + 1)],
                                         owq[1][:, cs:cs + cl], start=False, stop=False)
                        nc.tensor.matmul(p[:, 0:cl], ones_row_bf[:],
                                         obq[0:1, cs:cs + cl], start=False, stop=True)
                        if nci % 2 == 0:
                            nc.vector.tensor_copy(fst[:, cs:cs + cl], p[:, 0:cl])
                        else:
                            nc.scalar.copy(fst[:, cs:cs + cl], p[:, 0:cl])
                        nci += 1
                    nc.sync.dma_start(
                        logits[128 * ti:128 * (ti + 1), v0q:v0q + real],
                        fst[:, 0:real],
                    )

    nc.finalize()
    return nc


def kernel(**inputs):
    if "nc" not in _CACHE:
        _CACHE["nc"] = build()
    nc = _CACHE["nc"]

    per_core = ("tractovka", "context", "card", "prev_tokens")
    in_maps = []
    for core in range(NCORES):
        m = {}
        for k, v in inputs.items():
            v = np.asarray(v)
            if k in per_core:
                m[k] = np.ascontiguousarray(v[core * BL:(core + 1) * BL])
            else:
                m[k] = np.ascontiguousarray(v)
        in_maps.append(m)
    res = run_bass_kernel_spmd(nc, in_maps, list(range(NCORES)))
    out = np.concatenate(
        [res.results[i]["logits"].reshape(BL, S, V) for i in range(NCORES)], axis=0
    )
    return out



# revision 27
# speedup vs baseline: 1.4689x; 1.4689x over previous
"""Trainium2 Bass kernel for nn_EnhancedTarotInterpreter (dense transformer decoder).

Sharding: pure data parallel over batch (16 -> 8 cores x 2). Each core runs the
full model on its 2 batch elements; no collectives. Inside a core:

- Activations feature-major ("x.T": [d_chunk 128, tokens 2048]); residual/LN in
  f32r, matmul operands for the big GEMMs in bf16.
- All large weights are cast fp32->bf16 into internal DRAM with one SWDGE DMA
  each, then loaded transposed per layer via XBAR DMA-transpose (contiguous
  descriptors; the old strided loads generated 4-byte DMA packets).
- All bias / layernorm vectors are loaded with a handful of natural row-major
  DMAs and transposed on-chip with the PE (columns) or used as rows directly.
- Cross-attention memory has length 1 -> softmax is identity -> the whole block
  collapses to one bias vector per batch element (precomputed in the prologue).
- Self-attention: scores transposed [t, s] (K=32 matmuls), exp straight out of
  PSUM on ACT, causal mask only on the diagonal 128x128 block, AV flipped
  (out [s,33] bf16) with a ones-column in V so the denominator lands
  per-partition.
- LayerNorm feature-major: column stats via ones-matmul on PE, per-column
  affine via PE rank-1/rank-2 broadcasts into PSUM + two DVE passes.
- Final projection vocab-major: out[vocab 128, tokens 512] so out_b is a
  per-partition bias (fused into the PSUM evacuation); logits stored
  TRANSPOSED in bf16 ([VP2, S2]) and fixed up on the host.
"""

import sys

sys.path.insert(0, "/opt/trn_rl_repo")

import numpy as np

import concourse.bass as bass
import concourse.bacc as bacc
import concourse.mybir as mybir
import concourse.tile as tile
from concourse.bass_utils import run_bass_kernel_spmd

FP32 = mybir.dt.float32
FP32R = mybir.dt.float32r
BF16 = mybir.dt.bfloat16
I32 = mybir.dt.int32
AF = mybir.ActivationFunctionType
OP = mybir.AluOpType
AX = mybir.AxisListType

B, S, E, D, V, H, NL = 16, 1024, 64, 256, 10000, 8, 3
HD = D // H          # 32
FF = 4 * D           # 1024
NCORES = 8
BL = B // NCORES     # 2
S2 = BL * S          # 2048
VT = 79              # vocab tiles of 128
VP2 = VT * 128       # 10112
VSLAB = 1024         # vocab slab (8 tiles) for the final projection
ISCL = 1.0 / float(np.sqrt(HD))
EPS = 1e-5

_CACHE = {}


def _t_ap(dram, offset, pstep, pcount, fstep, fcount):
    """Manual 2D access pattern on a DRAM tensor: [partition, free]."""
    h = dram.tensor if hasattr(dram, "tensor") else dram
    if pcount == 1 and pstep == 0:
        pstep = 1
    return bass.AP(tensor=h, offset=offset, ap=[[pstep, pcount], [fstep, fcount]])


def build():
    nc = bacc.Bacc("TRN2", target_bir_lowering=False)

    # ---------------- DRAM I/O ----------------
    tract = nc.dram_tensor("tractovka", [BL, E], FP32, kind="ExternalInput")
    ctx = nc.dram_tensor("context", [BL, E], FP32, kind="ExternalInput")
    card = nc.dram_tensor("card", [BL, E], FP32, kind="ExternalInput")
    enc_w = nc.dram_tensor("enc_w", [3, D, E], FP32, kind="ExternalInput")
    enc_b = nc.dram_tensor("enc_b", [3, D], FP32, kind="ExternalInput")
    enc_ln_g = nc.dram_tensor("enc_ln_g", [3, D], FP32, kind="ExternalInput")
    enc_ln_b = nc.dram_tensor("enc_ln_b", [3, D], FP32, kind="ExternalInput")
    fusion_w = nc.dram_tensor("fusion_w", [D, 3 * D], FP32, kind="ExternalInput")
    fusion_b = nc.dram_tensor("fusion_b", [D], FP32, kind="ExternalInput")
    fusion_ln_g = nc.dram_tensor("fusion_ln_g", [D], FP32, kind="ExternalInput")
    fusion_ln_b = nc.dram_tensor("fusion_ln_b", [D], FP32, kind="ExternalInput")
    tok_emb = nc.dram_tensor("tok_emb", [V, D], FP32, kind="ExternalInput")
    pos_emb = nc.dram_tensor("pos_emb", [1024, D], FP32, kind="ExternalInput")
    sa_in_w = nc.dram_tensor("sa_in_w", [NL, 3 * D, D], FP32, kind="ExternalInput")
    sa_in_b = nc.dram_tensor("sa_in_b", [NL, 3 * D], FP32, kind="ExternalInput")
    sa_out_w = nc.dram_tensor("sa_out_w", [NL, D, D], FP32, kind="ExternalInput")
    sa_out_b = nc.dram_tensor("sa_out_b", [NL, D], FP32, kind="ExternalInput")
    ca_in_w = nc.dram_tensor("ca_in_w", [NL, 3 * D, D], FP32, kind="ExternalInput")
    ca_in_b = nc.dram_tensor("ca_in_b", [NL, 3 * D], FP32, kind="ExternalInput")
    ca_out_w = nc.dram_tensor("ca_out_w", [NL, D, D], FP32, kind="ExternalInput")
    ca_out_b = nc.dram_tensor("ca_out_b", [NL, D], FP32, kind="ExternalInput")
    ln_g = [nc.dram_tensor(f"ln{k}_g", [NL, D], FP32, kind="ExternalInput") for k in (1, 2, 3)]
    ln_b = [nc.dram_tensor(f"ln{k}_b", [NL, D], FP32, kind="ExternalInput") for k in (1, 2, 3)]
    ffn_w1 = nc.dram_tensor("ffn_w1", [NL, FF, D], FP32, kind="ExternalInput")
    ffn_b1 = nc.dram_tensor("ffn_b1", [NL, FF], FP32, kind="ExternalInput")
    ffn_w2 = nc.dram_tensor("ffn_w2", [NL, D, FF], FP32, kind="ExternalInput")
    ffn_b2 = nc.dram_tensor("ffn_b2", [NL, D], FP32, kind="ExternalInput")
    out_w = nc.dram_tensor("out_w", [V, D], FP32, kind="ExternalInput")
    out_b = nc.dram_tensor("out_b", [V], FP32, kind="ExternalInput")
    prev = nc.dram_tensor("prev_tokens", [BL, S], I32, kind="ExternalInput")

    logitsT = nc.dram_tensor("logitsT", [VP2, S2], BF16, kind="ExternalOutput")
    # internal bf16 weight copies (2D so XBAR slices are plain 2D APs)
    wq_bf = nc.dram_tensor("wq_bf", [NL * 3 * D, D], BF16)
    wo_bf = nc.dram_tensor("wo_bf", [NL * D, D], BF16)
    w1_bf = nc.dram_tensor("w1_bf", [NL * FF, D], BF16)
    w2_bf = nc.dram_tensor("w2_bf", [NL * D, FF], BF16)
    outw_bf = nc.dram_tensor("outw_bf", [VP2, D], BF16)

    from contextlib import ExitStack

    with tile.TileContext(nc) as tc:
        with ExitStack() as _es:
            P_const = _es.enter_context(tc.tile_pool(name="const", bufs=1))
            P_prm = _es.enter_context(tc.tile_pool(name="prm", bufs=1))
            P_w = _es.enter_context(tc.tile_pool(name="wgt", bufs=1))
            P_stage = _es.enter_context(tc.tile_pool(name="stage", bufs=2))
            P_x = _es.enter_context(tc.tile_pool(name="X", bufs=4))
            P_qk = _es.enter_context(tc.tile_pool(name="qk", bufs=2))
            P_vex = _es.enter_context(tc.tile_pool(name="vex", bufs=16))
            P_e = _es.enter_context(tc.tile_pool(name="e", bufs=1))
            P_otok = _es.enter_context(tc.tile_pool(name="otok", bufs=9))
            P_oT = _es.enter_context(tc.tile_pool(name="oT", bufs=2))
            P_h1 = _es.enter_context(tc.tile_pool(name="h1", bufs=8))
            P_t1 = _es.enter_context(tc.tile_pool(name="t1", bufs=3))
            P_rows = _es.enter_context(tc.tile_pool(name="rows", bufs=2))
            P_small = _es.enter_context(tc.tile_pool(name="small", bufs=4))
            P_fin = _es.enter_context(tc.tile_pool(name="fin", bufs=2))
            PS_st = _es.enter_context(tc.tile_pool(name="psst", bufs=2, space="PSUM"))
            PS_mm = _es.enter_context(tc.tile_pool(name="psmm", bufs=4, space="PSUM"))

            def mmtile(shape=None, dtype=FP32):
                return PS_mm.tile([128, 512] if shape is None else shape, dtype,
                                  tag="mm", name="mm")

            # ---------------- bf16 weight casts (SWDGE, issue first) -------
            nc.gpsimd.dma_start(wq_bf[:, :], _t_ap(sa_in_w, 0, D, NL * 3 * D, 1, D))
            nc.gpsimd.dma_start(wo_bf[:, :], _t_ap(sa_out_w, 0, D, NL * D, 1, D))
            nc.gpsimd.dma_start(w1_bf[:, :], _t_ap(ffn_w1, 0, D, NL * FF, 1, D))
            nc.gpsimd.dma_start(w2_bf[:, :], _t_ap(ffn_w2, 0, FF, NL * D, 1, FF))
            nc.gpsimd.dma_start(outw_bf[0:V, :], out_w[:])
            # (out_b handled as f32 natural loads below)

            # ---------------- constants ----------------
            ident_f = P_const.tile([128, 128], FP32)
            nc.gpsimd.memset(ident_f[:], 0.0)
            nc.gpsimd.affine_select(
                out=ident_f[:], in_=ident_f[:], compare_op=OP.not_equal, fill=1.0,
                base=0, pattern=[[-1, 128]], channel_multiplier=1,
            )
            ident_r = P_const.tile([128, 128], FP32R)
            nc.vector.tensor_copy(ident_r[:], ident_f[:])
            ident_bf = P_const.tile([128, 128], BF16)
            nc.vector.tensor_copy(ident_bf[:], ident_f[:])

            masktri_f = P_stage.tile([128, 128], FP32, tag="identstg", name="masktri_f", bufs=1)
            nc.gpsimd.memset(masktri_f[:], 1.0)
            nc.gpsimd.affine_select(
                out=masktri_f[:], in_=masktri_f[:], compare_op=OP.is_ge, fill=0.0,
                base=0, pattern=[[1, 128]], channel_multiplier=-1,
            )
            masktri = P_const.tile([128, 128], BF16)
            nc.vector.tensor_copy(masktri[:], masktri_f[:])

            ones_f = P_const.tile([128, 1], FP32)
            nc.vector.memset(ones_f[:], 1.0)
            ones_col = P_const.tile([128, 1], FP32R)       # [K=128, M=1] stats lhsT
            nc.vector.tensor_copy(ones_col[:], ones_f[:])
            onesr_f = P_stage.tile([1, 128], FP32, tag="onesrstg", name="onesr_f",
                                   bufs=1)
            nc.vector.memset(onesr_f[:], 1.0)
            ones_row = P_const.tile([1, 128], FP32R)       # rank-1 lhsT rows
            nc.vector.tensor_copy(ones_row[:], onesr_f[:])
            onesr512_f = P_stage.tile([1, 512], FP32, tag="onesr512stg",
                                       name="onesr512_f", bufs=1)
            nc.vector.memset(onesr512_f[:], 1.0)
            ones512_r = P_const.tile([1, 512], FP32R)
            nc.vector.tensor_copy(ones512_r[:], onesr512_f[:])

            eps2 = P_const.tile([BL, 1], FP32)
            nc.vector.memset(eps2[:], EPS)
            eps128 = P_const.tile([128, 1], FP32)
            nc.vector.memset(eps128[:], EPS)

            # ---------------- parameter loads (natural) + PE transposes ----
            _uid = [0]

            def ptile(shape, dtype, base):
                """Persistent param tile with a unique tag (never recycled)."""
                _uid[0] += 1
                return P_prm.tile(shape, dtype, tag=f"{base}{_uid[0]}",
                                  name=f"{base}{_uid[0]}")

            def nat_load(dram, offset, p, f, fstep=None, tag="prmld", bufs=4):
                t = P_prm.tile([p, f], FP32, tag=tag, name=tag, bufs=bufs)
                nc.sync.dma_start(t[:], _t_ap(dram, offset,
                                              f if fstep is None else fstep, p, 1, f))
                return t

            def cols_from_rows(dst, col0, src_sb, nr, nch, f0=0):
                """src_sb rows f32 -> dst[:, col0 + ch*nr + r] (from cols f0+...)."""
                for chk in range(nch):
                    tp = mmtile([128, nr], FP32)
                    nc.tensor.transpose(
                        tp[:],
                        src_sb[0:nr, f0 + 128 * chk:f0 + 128 * (chk + 1)],
                        ident_f[0:nr, 0:nr])
                    nc.vector.tensor_copy(
                        dst[:, col0 + nr * chk:col0 + nr * (chk + 1)], tp[:])

            def make_row(cols, idxs):
                """Build a [1, 128*len(idxs)] f32r row (base partition 0) from
                column-tile slices via PE transpose."""
                rt = P_rows.tile([1, 128 * len(idxs)], FP32R, tag="mkrow",
                                 name="mkrow", bufs=2)
                for n, (ct, ci) in enumerate(idxs):
                    tp = mmtile([1, 128], FP32)
                    nc.tensor.transpose(tp[:], ct[:, ci:ci + 1], ident_f[:])
                    nc.vector.tensor_copy(rt[0:1, 128 * n:128 * (n + 1)], tp[:])
                return rt

            # persistent: sa_in_b rows (needed per layer for vex bias packing)
            sab_sb = ptile([3, 3 * D], FP32, "sab")
            nc.sync.dma_start(sab_sb[:], _t_ap(sa_in_b, 0, 3 * D, 3, 1, 3 * D))

            qkcols = ptile([128, 12], FP32, "qkcols")       # col = 3*oc + li
            cols_from_rows(qkcols, 0, sab_sb, 3, 4)
            nc.vector.tensor_scalar(out=qkcols[:, 0:6], in0=qkcols[:, 0:6],
                                    scalar1=ISCL, scalar2=None, op0=OP.mult)
            obcols = ptile([128, 6], FP32, "obcols")        # col = 3*c + li
            sob_sb = nat_load(sa_out_b, 0, 3, D)
            cols_from_rows(obcols, 0, sob_sb, 3, 2)
            b1cols = ptile([128, 24], FP32, "b1cols")       # col = 3*hk + li
            for half in range(2):
                fh = nat_load(ffn_b1, 512 * half, 3, 512, fstep=FF)
                cols_from_rows(b1cols, 12 * half, fh, 3, 4)
            b2cols = ptile([128, 6], FP32, "b2cols")
            f2b_sb = nat_load(ffn_b2, 0, 3, D)
            cols_from_rows(b2cols, 0, f2b_sb, 3, 2)

            lngcol, lnbcol = [], []
            for k in range(3):
                g_sb = nat_load(ln_g[k], 0, 3, D)
                b_sb = nat_load(ln_b[k], 0, 3, D)
                gcol = ptile([128, 6], FP32, "lngcol")      # col = 3*c + li
                cols_from_rows(gcol, 0, g_sb, 3, 2)
                lngcol.append(gcol)
                bcol = ptile([128, 6], FP32, "lnbcol")
                cols_from_rows(bcol, 0, b_sb, 3, 2)
                lnbcol.append(bcol)

            # enc/fusion LN gamma/beta: [BL, D] bf16 broadcast tiles (SWDGE cast)
            def bcast2(dram, offset):
                t = ptile([BL, D], BF16, "gb2")
                nc.gpsimd.dma_start(t[:], _t_ap(dram, offset, 0, BL, 1, D))
                return t

            encg2 = [bcast2(enc_ln_g, i * D) for i in range(3)]
            encb2 = [bcast2(enc_ln_b, i * D) for i in range(3)]
            fusg2 = bcast2(fusion_ln_g, 0)
            fusb2 = bcast2(fusion_ln_b, 0)
            fusb_sb = ptile([1, D], FP32, "fusb")
            nc.sync.dma_start(fusb_sb[:], _t_ap(fusion_b, 0, 0, 1, 1, D))

            # bias rows for enc / cross-attn rank-1 matmuls -> column tiles
            vbcols = ptile([128, 6], FP32, "vbcols")        # sa_in_b v-part
            cols_from_rows(vbcols, 0, sab_sb, 3, 2, f0=2 * D)
            encbcols = ptile([128, 6], FP32, "encbcols")
            encb_sb = nat_load(enc_b, 0, 3, D)
            cols_from_rows(encbcols, 0, encb_sb, 3, 2)
            cavbcols = ptile([128, 6], FP32, "cavbcols")
            cab_sb = nat_load(ca_in_b, 0, 3, 3 * D)
            cols_from_rows(cavbcols, 0, cab_sb, 3, 2, f0=2 * D)
            cobcols = ptile([128, 6], FP32, "cobcols")
            cob_sb = nat_load(ca_out_b, 0, 3, D)
            cols_from_rows(cobcols, 0, cob_sb, 3, 2)

            # out_b -> biasT [128, VT] f32 (vocab-major bias columns)
            ob1_sb = nat_load(out_b, 0, 78, 128)
            ob2_sb = nat_load(out_b, 78 * 128, 1, 16)
            biasT = ptile([128, VT], FP32, "biasT")
            nc.vector.memset(biasT[:], 0.0)
            tpb = mmtile([128, 78], FP32)
            nc.tensor.transpose(tpb[:], ob1_sb[:], ident_f[0:78, 0:78])
            nc.vector.tensor_copy(biasT[:, 0:78], tpb[:])
            tpb2 = mmtile([16, 1], FP32)
            nc.tensor.transpose(tpb2[:], ob2_sb[:], ident_f[0:1, 0:1])
            nc.vector.tensor_copy(biasT[0:16, 78:79], tpb2[:])

            # enc_w -> ewT[i] [64, 256] f32r
            ewT = []
            for i in range(3):
                wt = ptile([E, D], FP32R, "ewT")
                for t_ in range(2):
                    nsb = nat_load(enc_w, i * D * E + 128 * t_ * E, 128, E)
                    tp = mmtile([E, 128], FP32)
                    nc.tensor.transpose(tp[:], nsb[:], ident_f[:])
                    nc.vector.tensor_copy(wt[:, 128 * t_:128 * (t_ + 1)], tp[:])
                ewT.append(wt)

            # fusion_w [D, 3D] staged natural; transposed per-chunk at use site
            fw_sb = [nat_load(fusion_w, 128 * c_ * 3 * D, 128, 3 * D)
                     for c_ in range(2)]

            def fw_chunk(k):
                wt = P_prm.tile([128, D], FP32R, tag="fwT", name="fwT", bufs=2)
                for c_ in range(2):
                    tp = mmtile([128, 128], FP32)
                    nc.tensor.transpose(
                        tp[:], fw_sb[c_][:, 128 * k:128 * (k + 1)], ident_f[:])
                    nc.vector.tensor_copy(wt[:, 128 * c_:128 * (c_ + 1)], tp[:])
                return wt

            # ca weights are transposed per layer inside the oca loop below
            def ca_wT(dram, base_off, tag):
                wt = [P_prm.tile([128, D], FP32R, tag=tag, name=tag, bufs=2)
                      for _ in range(2)]
                for d_ in range(2):
                    wsb = nat_load(dram, base_off + 128 * d_ * D, 128, D)
                    for c_ in range(2):
                        tp = mmtile([128, 128], FP32)
                        nc.tensor.transpose(
                            tp[:], wsb[:, 128 * c_:128 * (c_ + 1)], ident_f[:])
                        nc.vector.tensor_copy(wt[c_][:, 128 * d_:128 * (d_ + 1)], tp[:])
                return wt

            # ---------------- embedding -> xT ----------------
            xT = [P_x.tile([128, S2], FP32R, tag="X", name="xT") for _ in range(2)]
            for ti in range(16):
                b_, a_ = ti // 8, ti % 8
                idx_t = P_small.tile([128, 1], I32, tag="idx", name="idx")
                nc.sync.dma_start(idx_t[:], prev[b_, 128 * a_:128 * (a_ + 1)])
                emb_t = P_stage.tile([128, D], FP32, tag="emb", name="emb")
                nc.gpsimd.indirect_dma_start(
                    out=emb_t[:], out_offset=None, in_=tok_emb[:],
                    in_offset=bass.IndirectOffsetOnAxis(ap=idx_t[:, 0:1], axis=0),
                )
                pos_t = P_stage.tile([128, D], FP32, tag="pos", name="pos")
                nc.sync.dma_start(pos_t[:], pos_emb[128 * a_:128 * (a_ + 1), :])
                sum_t = P_stage.tile([128, D], FP32R, tag="esum", name="esum")
                nc.vector.tensor_tensor(out=sum_t[:], in0=emb_t[:], in1=pos_t[:],
                                        op=OP.add)
                for c in range(2):
                    pt = mmtile([128, 128], FP32R)
                    nc.tensor.transpose(pt[:], sum_t[:, 128 * c:128 * (c + 1)], ident_r[:])
                    nc.vector.tensor_copy(xT[c][:, 128 * ti:128 * (ti + 1)], pt[:])

            # ---------------- encoders / fusion / cross-attn vectors -------
            def token_ln_gelu(psum_ap, g2, b2, dst_ap):
                red = P_small.tile([BL, 1], FP32, tag="red", name="red")
                nc.vector.tensor_reduce(out=red[:], in_=psum_ap, axis=AX.X, op=OP.add)
                m = P_small.tile([BL, 1], FP32, tag="m", name="m")
                nc.vector.tensor_scalar(out=m[:], in0=red[:], scalar1=1.0 / D,
                                        scalar2=None, op0=OP.mult)
                xc = P_stage.tile([BL, D], FP32, tag="xc", name="xc", bufs=1)
                nc.vector.tensor_scalar(out=xc[:], in0=psum_ap, scalar1=m[:],
                                        scalar2=None, op0=OP.subtract)
                sq = P_stage.tile([BL, D], FP32, tag="sq", name="sq", bufs=1)
                nc.vector.tensor_tensor(out=sq[:], in0=xc[:], in1=xc[:], op=OP.mult)
                red2 = P_small.tile([BL, 1], FP32, tag="red2", name="red2")
                nc.vector.tensor_reduce(out=red2[:], in_=sq[:], axis=AX.X, op=OP.add)
                var = P_small.tile([BL, 1], FP32, tag="var", name="var")
                nc.vector.tensor_scalar(out=var[:], in0=red2[:], scalar1=1.0 / D,
                                        scalar2=None, op0=OP.mult)
                std = P_small.tile([BL, 1], FP32, tag="std", name="std")
                nc.scalar.activation(std[:], var[:], AF.Sqrt, bias=eps2[:], scale=1.0)
                rstd = P_small.tile([BL, 1], FP32, tag="rstd", name="rstd")
                nc.vector.reciprocal(rstd[:], std[:])
                xn = P_stage.tile([BL, D], FP32, tag="xn", name="xn", bufs=1)
                nc.vector.tensor_scalar(out=xn[:], in0=xc[:], scalar1=rstd[:],
                                        scalar2=None, op0=OP.mult)
                nc.vector.tensor_tensor(out=xn[:], in0=xn[:], in1=g2[:], op=OP.mult)
                nc.vector.tensor_tensor(out=xn[:], in0=xn[:], in1=b2[:], op=OP.add)
                nc.scalar.activation(dst_ap, xn[:], AF.Gelu)

            def small_transposes(src_fp32, n_chunks, tag):
                outs = []
                for k in range(n_chunks):
                    pt = mmtile([128, BL], FP32)
                    nc.tensor.transpose(
                        pt[:], src_fp32[0:BL, 128 * k:128 * (k + 1)],
                        ident_f[0:BL, 0:BL]
                    )
                    st = P_small.tile([128, BL], FP32R, tag=tag, name=tag, bufs=8)
                    nc.vector.tensor_copy(st[:], pt[:])
                    outs.append(st)
                return outs

            cat = P_stage.tile([BL, 3 * D], FP32, tag="cat", name="cat", bufs=1)
            for i, src in enumerate((tract, ctx, card)):
                src_sb = P_stage.tile([BL, E], FP32, tag="encin", name="encin", bufs=1)
                nc.sync.dma_start(src_sb[:], src[:])
                src_r = P_stage.tile([BL, E], FP32R, tag="encinr", name="encinr", bufs=1)
                nc.vector.tensor_copy(src_r[:], src_sb[:])
                inT = mmtile([E, BL], FP32R)
                nc.tensor.transpose(inT[:], src_r[:], ident_r[0:BL, 0:BL])
                inT_sb = P_small.tile([E, BL], FP32R, tag="encT", name="encT", bufs=3)
                nc.vector.tensor_copy(inT_sb[:], inT[:])
                ebr = make_row(encbcols, [(encbcols, 3 * 0 + i), (encbcols, 3 * 1 + i)])
                pe_ = mmtile([BL, D])
                nc.tensor.matmul(pe_[:], inT_sb[:], ewT[i][:], start=True, stop=False)
                nc.tensor.matmul(pe_[:], ones_row[0:1, 0:BL], ebr[:],
                                 start=False, stop=True)
                token_ln_gelu(pe_[:], encg2[i], encb2[i],
                              cat[:, D * i:D * (i + 1)])

            catT = small_transposes(cat, 6, "catT")
            pf = mmtile([BL, D])
            for k in range(6):
                fwk = fw_chunk(k)
                nc.tensor.matmul(pf[:], catT[k][:], fwk[:], start=(k == 0), stop=False)
            fusb_r = P_rows.tile([1, D], FP32R, tag="mkrow", name="fusbr", bufs=2)
            nc.vector.tensor_copy(fusb_r[:], fusb_sb[:])
            nc.tensor.matmul(pf[:], ones_row[0:1, 0:BL], fusb_r[:],
                             start=False, stop=True)
            mem = P_stage.tile([BL, D], FP32, tag="encout", name="mem", bufs=1)
            token_ln_gelu(pf[:], fusg2, fusb2, mem[:])

            memT = small_transposes(mem, 2, "memT")
            oca = []
            for i in range(NL):
                cav = ca_wT(ca_in_w, i * 3 * D * D + 2 * D * D, "cavT")
                pv = mmtile([BL, D])
                for c in range(2):
                    nc.tensor.matmul(pv[:], memT[c][:], cav[c][:],
                                     start=(c == 0), stop=False)
                vbr = make_row(cavbcols, [(cavbcols, 3 * 0 + i), (cavbcols, 3 * 1 + i)])
                nc.tensor.matmul(pv[:], ones_row[0:1, 0:BL], vbr[:],
                                 start=False, stop=True)
                v_sb = P_stage.tile([BL, D], FP32, tag="cav", name="cav", bufs=1)
                nc.vector.tensor_copy(v_sb[:], pv[:])
                vT = small_transposes(v_sb, 2, "vT")
                cawo = ca_wT(ca_out_w, i * D * D, "cawoT")
                po = mmtile([BL, D])
                for c in range(2):
                    nc.tensor.matmul(po[:], vT[c][:], cawo[c][:],
                                     start=(c == 0), stop=False)
                obr = make_row(cobcols, [(cobcols, 3 * 0 + i), (cobcols, 3 * 1 + i)])
                nc.tensor.matmul(po[:], ones_row[0:1, 0:BL], obr[:],
                                 start=False, stop=True)
                o_sb = P_stage.tile([BL, D], FP32, tag="cao", name="cao", bufs=1)
                nc.vector.tensor_copy(o_sb[:], po[:])
                ocT = small_transposes(o_sb, 2, "ocT")
                ocf = []
                for c in range(2):
                    t = P_small.tile([128, BL], FP32, tag="oca", name="oca", bufs=6)
                    nc.vector.tensor_copy(t[:], ocT[c][:].bitcast(FP32))
                    ocf.append(t)
                oca.append(ocf)

            # ---------------- feature-major LayerNorm ----------------
            def layer_norm(xr, k, li):
                gc = lngcol[k]
                g_row, nb_row = [], []
                for c in range(2):
                    tpg = mmtile([1, 128], FP32)
                    nc.tensor.transpose(
                        tpg[:], gc[:, 3 * c + li:3 * c + li + 1], ident_f[:])
                    gr = P_rows.tile([1, 128], FP32R, tag="lnrow", name="lnrow",
                                     bufs=4)
                    nc.vector.tensor_copy(gr[:], tpg[:])
                    g_row.append(gr)
                    tpb_ = mmtile([1, 128], FP32)
                    nc.tensor.transpose(
                        tpb_[:], lnbcol[k][:, 3 * c + li:3 * c + li + 1], ident_f[:])
                    nr_ = P_rows.tile([1, 128], FP32R, tag="lnrow", name="lnrow",
                                      bufs=4)
                    nc.vector.tensor_scalar(out=nr_[:], in0=tpb_[:], scalar1=-1.0,
                                            scalar2=None, op0=OP.mult)
                    nb_row.append(nr_)
                m4 = P_rows.tile([128, 512], FP32, tag="m4", name="m4", bufs=1)
                e24 = P_rows.tile([128, 512], FP32, tag="e24", name="e24", bufs=1)
                msq4 = P_t1.tile([128, 512], FP32, tag="t1", name="msq4")
                for j in range(4):
                    sl = slice(512 * j, 512 * (j + 1))
                    xsq = [P_t1.tile([128, 512], FP32R, tag="t1", name="xsq")
                           for _ in range(2)]
                    for c in range(2):
                        nc.vector.tensor_tensor(out=xsq[c][:], in0=xr[c][:, sl],
                                                in1=xr[c][:, sl], op=OP.mult)
                    st_ = mmtile()
                    nc.tensor.matmul(st_[0:1, :], ones_col[:], xr[0][:, sl],
                                     start=True, stop=False)
                    nc.tensor.matmul(st_[0:1, :], ones_col[:], xr[1][:, sl],
                                     start=False, stop=True)
                    st2_ = mmtile()
                    nc.tensor.matmul(st2_[0:1, :], ones_col[:], xsq[0][:],
                                     start=True, stop=False)
                    nc.tensor.matmul(st2_[0:1, :], ones_col[:], xsq[1][:],
                                     start=False, stop=True)
                    nc.vector.tensor_scalar(out=m4[32 * j:32 * j + 1, :], in0=st_[0:1, :],
                                            scalar1=1.0 / D, scalar2=None, op0=OP.mult)
                    nc.scalar.mul(e24[32 * j:32 * j + 1, :], st2_[0:1, :], 1.0 / D)
                nc.scalar.activation(msq4[:], m4[:], AF.Square)
                nc.vector.tensor_tensor(out=e24[:], in0=e24[:], in1=msq4[:],
                                        op=OP.subtract)
                nc.scalar.activation(e24[:], e24[:], AF.Sqrt, bias=eps128[:], scale=1.0)
                nc.vector.reciprocal(e24[:], e24[:])   # e24 now holds rstd rows
                xo = [P_x.tile([128, S2], FP32R, tag="X", name="xo") for _ in range(2)]
                for j in range(4):
                    sl = slice(512 * j, 512 * (j + 1))
                    r_r = P_rows.tile([1, 512], FP32R, tag="rr", name="rr", bufs=1)
                    nc.vector.tensor_copy(r_r[:], e24[32 * j:32 * j + 1, :])
                    c_r = P_rows.tile([1, 512], FP32R, tag="cr", name="cr", bufs=1)
                    nc.vector.tensor_tensor(out=c_r[:], in0=m4[32 * j:32 * j + 1, :],
                                            in1=e24[32 * j:32 * j + 1, :], op=OP.mult)
                    rb = mmtile()
                    nc.tensor.matmul(rb[:], ones_row[0:1, 0:128], r_r[:],
                                     start=True, stop=True)
                    for c in range(2):
                        db = mmtile()
                        nc.tensor.matmul(db[:], g_row[c][:], c_r[:],
                                         start=True, stop=False)
                        nc.tensor.matmul(db[:], nb_row[c][:], ones512_r[:],
                                         start=False, stop=True)
                        t1 = P_t1.tile([128, 512], FP32, tag="t1", name="t1")
                        nc.vector.tensor_tensor(out=t1[:], in0=xr[c][:, sl], in1=rb[:],
                                                op=OP.mult)
                        nc.vector.scalar_tensor_tensor(
                            out=xo[c][:, sl], in0=t1[:], scalar=gc[:, 3 * c + li:3 * c + li + 1],
                            in1=db[:], op0=OP.mult, op1=OP.subtract,
                        )
                return xo

            # ---------------- decoder layers ----------------
            x = xT
            for li in range(NL):
                # -- XBAR transposed weight loads (bf16, double-buffered) --
                wInT = [P_w.tile([128, 2 * D], BF16, tag=f"wInT{c}", name=f"wInT{c}",
                                 bufs=2) for c in range(2)]
                wvT = [P_w.tile([128, D], BF16, tag=f"wvT{c}", name=f"wvT{c}",
                                bufs=2) for c in range(2)]
                woT = [P_w.tile([128, D], BF16, tag=f"woT{c}", name=f"woT{c}",
                                bufs=2) for c in range(2)]
                w1T = [P_w.tile([128, FF], BF16, tag=f"w1T{c}", name=f"w1T{c}",
                                bufs=2) for c in range(2)]
                w2T = [P_w.tile([128, D], BF16, tag=f"w2T{k}", name=f"w2T{k}",
                                bufs=1) for k in range(8)]
                for c in range(2):
                    nc.sync.dma_start(wInT[c][:],
                                      wq_bf[li * 3 * D:li * 3 * D + 2 * D,
                                            128 * c:128 * (c + 1)], transpose=True)
                    nc.sync.dma_start(wvT[c][:],
                                      wq_bf[li * 3 * D + 2 * D:(li + 1) * 3 * D,
                                            128 * c:128 * (c + 1)], transpose=True)
                    nc.sync.dma_start(woT[c][:],
                                      wo_bf[li * D:(li + 1) * D,
                                            128 * c:128 * (c + 1)], transpose=True)
                    nc.sync.dma_start(w1T[c][:],
                                      w1_bf[li * FF:(li + 1) * FF,
                                            128 * c:128 * (c + 1)], transpose=True)
                for k in range(8):
                    nc.sync.dma_start(w2T[k][:],
                                      w2_bf[li * D:(li + 1) * D,
                                            128 * k:128 * (k + 1)], transpose=True)

                # wvxT [128, 264]: v-weights in 33-stride head packing + bias row
                wvxT = [P_w.tile([128, 264], BF16, tag=f"wvxT{c}", name=f"wvxT{c}",
                                 bufs=2) for c in range(2)]
                for c in range(2):
                    nc.gpsimd.memset(wvxT[c][:], 0.0)
                    for h in range(H):
                        nc.gpsimd.tensor_copy(wvxT[c][:, 33 * h:33 * h + 32],
                                              wvT[c][:, 32 * h:32 * h + 32])
                vrow = make_row(vbcols, [(vbcols, 3 * 0 + li), (vbcols, 3 * 1 + li)])
                bx_f = P_rows.tile([1, 264], FP32, tag="bxf", name="bxf", bufs=1)
                nc.vector.memset(bx_f[:], 1.0)
                for h in range(H):
                    nc.vector.tensor_copy(bx_f[0:1, 33 * h:33 * h + 32],
                                          vrow[0:1, 32 * h:32 * h + 32].bitcast(FP32))
                bx_r = P_rows.tile([1, 264], FP32R, tag="bxr", name="bxr", bufs=1)
                nc.vector.tensor_copy(bx_r[:], bx_f[:])

                inb = [qkcols[:, 3 * oc + li:3 * oc + li + 1] for oc in range(4)]
                ob_col = [obcols[:, 3 * c + li:3 * c + li + 1] for c in range(2)]
                b1_col = [b1cols[:, 3 * hk + li:3 * hk + li + 1] for hk in range(8)]
                b2_col = [b2cols[:, 3 * c + li:3 * c + li + 1] for c in range(2)]

                # -- bf16 copy of x for the GEMMs --
                xb = [P_qk.tile([128, S2], BF16, tag="xb", name="xb") for _ in range(2)]
                for c in range(2):
                    nc.gpsimd.tensor_copy(xb[c][:], x[c][:])

                # --- q,k projections (bf16; q pre-scaled by 1/sqrt(HD)) ---
                qT = [P_qk.tile([128, S2], BF16, tag="qT", name="qT") for _ in range(2)]
                kT = [P_qk.tile([128, S2], BF16, tag="kT", name="kT") for _ in range(2)]
                for oc in range(4):
                    dst = qT[oc] if oc < 2 else kT[oc - 2]
                    for j in range(4):
                        sl = slice(512 * j, 512 * (j + 1))
                        p = mmtile()
                        nc.tensor.matmul(p[:], wInT[0][:, 128 * oc:128 * (oc + 1)],
                                         xb[0][:, sl], start=True, stop=False)
                        nc.tensor.matmul(p[:], wInT[1][:, 128 * oc:128 * (oc + 1)],
                                         xb[1][:, sl], start=False, stop=True)
                        if oc < 2:
                            nc.vector.tensor_scalar(out=dst[:, sl], in0=p[:],
                                                    scalar1=inb[oc], scalar2=ISCL,
                                                    op0=OP.add, op1=OP.mult)
                        else:
                            nc.vector.tensor_scalar(out=dst[:, sl], in0=p[:],
                                                    scalar1=inb[oc], scalar2=None,
                                                    op0=OP.add)

                # --- v_ext [t, 264] bf16 ---
                vex = []
                for ti in range(16):
                    p = mmtile()
                    nc.tensor.matmul(p[:, 0:264], xb[0][:, 128 * ti:128 * (ti + 1)],
                                     wvxT[0][:], start=True, stop=False)
                    nc.tensor.matmul(p[:, 0:264], xb[1][:, 128 * ti:128 * (ti + 1)],
                                     wvxT[1][:], start=False, stop=False)
                    nc.tensor.matmul(p[:, 0:264], ones_row[0:1, 0:128], bx_r[:],
                                     start=False, stop=True)
                    vt_ = P_vex.tile([128, 264], BF16, tag="vex", name="vex")
                    nc.vector.tensor_copy(vt_[:], p[:, 0:264])
                    vex.append(vt_)

                # --- attention ---
                oT = [P_oT.tile([128, S2], BF16, tag="oT", name="oT") for _ in range(2)]
                for b_ in range(BL):
                    otoks = [P_otok.tile([128, 256], BF16, tag="otok", name="otok")
                             for _ in range(8)]
                    for h in range(H):
                        ch, po = h // 4, (h % 4) * 32
                        et = []
                        for a in range(8):
                            s0 = 128 * a
                            stp = PS_st.tile([128, 1024], FP32, tag="st", name="st")
                            breaks = [s0, 512, 1024] if s0 < 512 else [s0, 1024]
                            for cs, ce in zip(breaks[:-1], breaks[1:]):
                                nc.tensor.matmul(
                                    stp[:, cs:ce],
                                    kT[ch][po:po + 32, S * b_ + s0:S * b_ + s0 + 128],
                                    qT[ch][po:po + 32, S * b_ + cs:S * b_ + ce],
                                    start=True, stop=True,
                                    tile_position=(po, 0),
                                )
                            e_a = P_e.tile([128, 1024 - s0], BF16, tag=f"e{a}",
                                           name=f"e{a}")
                            nc.scalar.activation(e_a[:], stp[:, s0:1024], AF.Exp)
                            nc.vector.tensor_tensor(out=e_a[:, 0:128], in0=e_a[:, 0:128],
                                                    in1=masktri[:], op=OP.mult)
                            et.append(e_a)
                        for si in range(8):
                            pav = mmtile([128, 33])
                            for a in range(si + 1):
                                nc.tensor.matmul(
                                    pav[:],
                                    et[a][:, 128 * (si - a):128 * (si - a) + 128],
                                    vex[8 * b_ + a][:, 33 * h:33 * h + 33],
                                    start=(a == 0), stop=(a == si),
                                )
                            rr = P_small.tile([128, 1], FP32, tag="avrr", name="avrr", bufs=8)
                            nc.vector.reciprocal(rr[:], pav[:, 32:33])
                            nc.vector.tensor_scalar(
                                out=otoks[si][:, 32 * h:32 * h + 32], in0=pav[:, 0:32],
                                scalar1=rr[:], scalar2=None, op0=OP.mult,
                            )
                    for si in range(8):
                        for c in range(2):
                            pt = mmtile([128, 128], BF16)
                            nc.tensor.transpose(
                                pt[:], otoks[si][:, 128 * c:128 * (c + 1)], ident_bf[:]
                            )
                            nc.vector.tensor_copy(
                                oT[c][:, S * b_ + 128 * si:S * b_ + 128 * (si + 1)],
                                pt[:],
                            )

                # --- out_proj + residual -> xr1, ln1 -> x1 ---
                xr1 = [P_x.tile([128, S2], FP32R, tag="X", name="xr1") for _ in range(2)]
                for c in range(2):
                    for j in range(4):
                        sl = slice(512 * j, 512 * (j + 1))
                        p = mmtile()
                        nc.tensor.matmul(p[:], woT[0][:, 128 * c:128 * (c + 1)],
                                         oT[0][:, sl], start=True, stop=False)
                        nc.tensor.matmul(p[:], woT[1][:, 128 * c:128 * (c + 1)],
                                         oT[1][:, sl], start=False, stop=True)
                        nc.vector.scalar_tensor_tensor(
                            out=xr1[c][:, sl], in0=p[:], scalar=ob_col[c],
                            in1=x[c][:, sl], op0=OP.add, op1=OP.add,
                        )
                x1 = layer_norm(xr1, 0, li)

                # --- cross-attention add -> xr2, ln2 -> x2 ---
                xr2 = [P_x.tile([128, S2], FP32R, tag="X", name="xr2") for _ in range(2)]
                for c in range(2):
                    for b_ in range(BL):
                        sl = slice(S * b_, S * (b_ + 1))
                        nc.vector.tensor_scalar(
                            out=xr2[c][:, sl], in0=x1[c][:, sl],
                            scalar1=oca[li][c][:, b_:b_ + 1], scalar2=None, op0=OP.add,
                        )
                x2 = layer_norm(xr2, 1, li)

                # --- FFN -> xr3, ln3 -> x ---
                x2b = [P_qk.tile([128, S2], BF16, tag="xb", name="x2b") for _ in range(2)]
                for c in range(2):
                    nc.gpsimd.tensor_copy(x2b[c][:], x2[c][:])
                xr3 = [P_x.tile([128, S2], FP32R, tag="X", name="xr3") for _ in range(2)]
                for j in range(4):
                    sl = slice(512 * j, 512 * (j + 1))
                    h1t = []
                    for hk in range(8):
                        p = mmtile()
                        nc.tensor.matmul(p[:], w1T[0][:, 128 * hk:128 * (hk + 1)],
                                         x2b[0][:, sl], start=True, stop=False)
                        nc.tensor.matmul(p[:], w1T[1][:, 128 * hk:128 * (hk + 1)],
                                         x2b[1][:, sl], start=False, stop=True)
                        ht = P_h1.tile([128, 512], BF16, tag="h1", name="h1")
                        nc.scalar.activation(ht[:], p[:], AF.Relu, bias=b1_col[hk],
                                             scale=1.0)
                        h1t.append(ht)
                    for c in range(2):
                        p = mmtile()
                        for k in range(8):
                            nc.tensor.matmul(p[:], w2T[k][:, 128 * c:128 * (c + 1)],
                                             h1t[k][:], start=(k == 0), stop=(k == 7))
                        nc.vector.scalar_tensor_tensor(
                            out=xr3[c][:, sl], in0=p[:], scalar=b2_col[c],
                            in1=x2[c][:, sl], op0=OP.add, op1=OP.add,
                        )
                x = layer_norm(xr3, 2, li)

            # ---------------- final projection (vocab-major, bf16) ----------
            xbf = [P_qk.tile([128, S2], BF16, tag="xb", name="xbf") for _ in range(2)]
            for c in range(2):
                nc.gpsimd.tensor_copy(xbf[c][:], x[c][:])
            slab_edges = list(range(0, VP2, VSLAB)) + [VP2]  # 7x1280 + 1152
            nci = 0
            for vq in range(len(slab_edges) - 1):
                v0q, v1q = slab_edges[vq], slab_edges[vq + 1]
                vw = v1q - v0q
                owq = [P_w.tile([128, VSLAB], BF16, tag=f"owq{c}", name=f"owq{c}",
                                bufs=2) for c in range(2)]
                for c in range(2):
                    nc.sync.dma_start(owq[c][:, 0:vw],
                                      outw_bf[v0q:v1q, 128 * c:128 * (c + 1)],
                                      transpose=True)
                for vtl in range(vw // 128):
                    vt_g = v0q // 128 + vtl
                    for ts in range(4):
                        sl = slice(512 * ts, 512 * (ts + 1))
                        fin = P_fin.tile([128, 512], BF16, tag="fin", name="fin")
                        p = mmtile()
                        nc.tensor.matmul(p[:], owq[0][:, 128 * vtl:128 * (vtl + 1)],
                                         xbf[0][:, sl], start=True, stop=False)
                        nc.tensor.matmul(p[:], owq[1][:, 128 * vtl:128 * (vtl + 1)],
                                         xbf[1][:, sl], start=False, stop=True)
                        if nci % 2 == 0:
                            nc.vector.tensor_scalar(
                                out=fin[:], in0=p[:],
                                scalar1=biasT[:, vt_g:vt_g + 1], scalar2=None,
                                op0=OP.add)
                        else:
                            nc.scalar.activation(fin[:], p[:], AF.Identity,
                                                 bias=biasT[:, vt_g:vt_g + 1],
                                                 scale=1.0)
                        nci += 1
                        nc.sync.dma_start(
                            logitsT[128 * vt_g:128 * (vt_g + 1), sl], fin[:])

    nc.finalize()
    return nc


def kernel(**inputs):
    if "nc" not in _CACHE:
        _CACHE["nc"] = build()
    nc = _CACHE["nc"]

    per_core = ("tractovka", "context", "card", "prev_tokens")
    in_maps = []
    for core in range(NCORES):
        m = {}
        for k, v in inputs.items():
            v = np.asarray(v)
            if k in per_core:
                m[k] = np.ascontiguousarray(v[core * BL:(core + 1) * BL])
            else:
                m[k] = np.ascontiguousarray(v)
        in_maps.append(m)
    res = run_bass_kernel_spmd(nc, in_maps, list(range(NCORES)))
    outs = []
    for i in range(NCORES):
        lt = np.asarray(res.results[i]["logitsT"])  # [VP2, S2] bf16
        outs.append(lt[:V].T.astype(np.float32).reshape(BL, S, V))
    return np.concatenate(outs, axis=0)
